# revision 17
# baseline (speedup 1.0000x reference)
"""Trainium2 Bass kernel for nn_CrossAttention (dense_transformer).

Sharding: data-parallel over batch B=8 across 8 NeuronCores (1 sample
per core). BatchNorm uses batch statistics, so per-channel partial
[sum, sumsq] are all-reduced across cores ([128,2] f32 payload, 2x).

Per-core layout: activations [C=128 partitions, N=H*W=2304 free].
Matmul dtype strategy: everything except the qh projections / pred
head runs bf16 x bf16 (inputs are cast to bf16 on the host, halving
the input DMA stream that dominates startup and cross-core skew; yp,
qh, pad, conv weights, v^T, exp'd energies are bf16 on device). bf16
stationaries enable the PE's fast-weight-load path (~100ns vs ~220ns
f32r per 128-col LDWEIGHTS) and 1 cycle/row at any free size. PSUM
accumulation is fp32 throughout; the xp/h2 residual paths stay fp32.

Attention in energy-transposed layout with algebraically folded
projections (saves the k-projection and all per-block transposes):
  energy^T[m,q] = sum_c yp[c,m] * qh[c,q],  qh = (Wq^T Wk)^T @ h
  v^T[mo]       = matmul(lhsT=yp[:,mo-chunk], rhs=gamma*Wv^T)
Softmax (no max-subtraction needed: |energy| <= ~10) reduces over m =
partitions: exp'd energies are pair-summed (two m-chunks share one
2-bank PSUM tile so each Exp activation covers both via a strided AP),
tree-folded on DVE/GPSIMD, and one ones-matmul per q-superblock does
the partition sum, arriving pre-broadcast across partitions; 1/colsum
is applied after the v-contraction (reciprocal_approx_fast).

conv3x3 = 9 shifted-window matmuls over a zero-padded [128,50,50]
bf16 buffer. The attention inner loop is Scalar(Exp)-bound (~1.03us
per pair-step vs ~0.82us of matmul), so conv rowchunk k-2's taps are
interleaved one-per-pair-step into attention superblock k — the conv
rides in the PE bubbles and only rowchunks 3,4 remain after the loop.
BN sumsq runs on DVE (affine_mul_reduce), keeping Scalar for Exp.

A warmup AllReduce (unread result) fires at kernel start: it absorbs
the cross-core launch skew and the CC-core cold-start. pred_b is
folded into the host-side gather.
"""

import sys

sys.path.insert(0, "/opt/trn_rl_repo")

import numpy as np

_NC_CACHE = {}

B, CIN, C, H, W = 8, 256, 128, 48, 48
N = H * W  # 2304
P = 128
NKO = CIN // P  # 2
NMO = N // P  # 18
# q superblocks: row-aligned chunks (48-col rows); 480 = 10 rows
QCH = [(0, 480), (480, 480), (960, 480), (1440, 480), (1920, 384)]
ROWCH = [(0, 10), (10, 10), (20, 10), (30, 10), (40, 8)]
# stage-A load/projection chunks: finer at the start so the first
# matmul fires early in the input DMA stream
LCH = [(0, 256), (256, 256), (512, 448), (960, 480), (1440, 480), (1920, 384)]
NSTAT = float(B * N)  # BN stat count over (B,H,W)
EPS = 1e-5


def _build(variant="default"):
    """variant: 'default' = 8-core w/ collectives; 'sim' = single-core,
    collectives replaced by DMA copy (for TimelineSim profiling)."""
    key = f"nc_{variant}"
    if key in _NC_CACHE:
        return _NC_CACHE[key]

    import concourse.mybir as mybir
    import concourse.tile as tile
    from concourse import bacc
    from contextlib import ExitStack

    F32 = mybir.dt.float32
    F32R = mybir.dt.float32r
    BF16 = mybir.dt.bfloat16
    AF = mybir.ActivationFunctionType
    ALU = mybir.AluOpType
    AX = mybir.AxisListType

    sim = variant == "sim"
    nc = bacc.Bacc(
        "TRN2", target_bir_lowering=False, debug=False,
        num_devices=1 if sim else 8,
    )

    # ---- DRAM I/O ----
    d_x = nc.dram_tensor("x", [CIN, N], BF16, kind="ExternalInput")
    d_y = nc.dram_tensor("y", [CIN, N], BF16, kind="ExternalInput")
    d_w_inT = nc.dram_tensor("w_inT", [P, NKO, P], BF16, kind="ExternalInput")
    d_b_in = nc.dram_tensor("b_in", [P, 1], F32, kind="ExternalInput")
    d_A1 = nc.dram_tensor("A1", [P, P], F32R, kind="ExternalInput")
    d_gwv1T = nc.dram_tensor("gwv1T", [P, P], BF16, kind="ExternalInput")
    d_A2 = nc.dram_tensor("A2", [P, P], F32R, kind="ExternalInput")
    d_gwv2T = nc.dram_tensor("gwv2T", [P, P], BF16, kind="ExternalInput")
    d_w1T = nc.dram_tensor("w1T", [P, 9, P], BF16, kind="ExternalInput")
    d_bn1s = nc.dram_tensor("bn1s", [P, 1], F32, kind="ExternalInput")
    d_bn1b = nc.dram_tensor("bn1b", [P, 1], F32, kind="ExternalInput")
    d_w2T = nc.dram_tensor("w2T", [P, 9, P], BF16, kind="ExternalInput")
    d_bn2s = nc.dram_tensor("bn2s", [P, 1], F32, kind="ExternalInput")
    d_bn2b = nc.dram_tensor("bn2b", [P, 1], F32, kind="ExternalInput")
    d_predT = nc.dram_tensor("predT", [P, P], F32R, kind="ExternalInput")
    d_out = nc.dram_tensor("out", [1, N], F32, kind="ExternalOutput")

    with tile.TileContext(nc) as tc, ExitStack() as ctx:
        wgt = ctx.enter_context(tc.tile_pool(name="wgt", bufs=1))
        act = ctx.enter_context(tc.tile_pool(name="act", bufs=1))
        ew = ctx.enter_context(tc.tile_pool(name="ew", bufs=1))
        eeP = ctx.enter_context(tc.tile_pool(name="eeP", bufs=6))
        load = ctx.enter_context(tc.tile_pool(name="load", bufs=12))
        dram = ctx.enter_context(tc.tile_pool(name="dram", bufs=1, space="DRAM"))
        pE = ctx.enter_context(tc.tile_pool(name="pE", bufs=2, space="PSUM"))
        pO = ctx.enter_context(tc.tile_pool(name="pO", bufs=2, space="PSUM"))
        pS = ctx.enter_context(tc.tile_pool(name="pS", bufs=1, space="PSUM"))
        pM = ctx.enter_context(tc.tile_pool(name="pM", bufs=1, space="PSUM"))

        # ---------- warmup AllReduce: unread result; absorbs launch skew
        # and CC-core cold start while the input DMAs stream ----------
        if not sim:
            cc_w_in = dram.tile([P, 2], F32, tag="cc_w_in")
            cc_w_out = dram.tile([P, 2], F32, tag="cc_w_out")
            nc.gpsimd.collective_compute(
                "AllReduce",
                mybir.AluOpType.add,
                replica_groups=[list(range(8))],
                ins=[cc_w_in[:].opt()],
                outs=[cc_w_out[:].opt()],
            )

        # ---------- weights (direct DMA) ----------
        def load_w(dsrc, shape, tag, dtype=F32R):
            t = wgt.tile(shape, dtype, tag=tag)
            nc.sync.dma_start(t[:], dsrc[...])
            return t

        # DMA emission order controls queue priority: only the weights
        # needed for the input projections go ahead of the x/y stream;
        # everything else is enqueued between/after the input chunks.
        w_inT_r = load_w(d_w_inT, [P, NKO, P], "w_inT_r", BF16)
        b_in = load_w(d_b_in, [P, 1], "b_in", F32)

        ones_b = wgt.tile([P, P], BF16, tag="ones_b")
        nc.gpsimd.memset(ones_b[:], 1.0)

        zrow = wgt.tile([P, W + 2], BF16, tag="zrow")
        nc.gpsimd.memset(zrow[:], 0.0)

        # ---------- stage A: load x,y per chunk (interleaved) so the
        # first projections start early; DMA tail overlaps attn1.
        # yp is written bf16 (energy/vT stationary); xp stays f32r
        # (qh1 moving operand + attn1 residual).
        xp = act.tile([P, N], F32R, tag="tagA")
        yp = act.tile([P, N], BF16, tag="tagB")
        late_w = {}
        for ci, (q0, qn) in enumerate(LCH):
            for ti, (dsrc, dst) in enumerate(((d_x, xp), (d_y, yp))):
                rr = [
                    load.tile([P, 480], BF16, tag="in_r", name=f"ld{ci}_{ti}_{ko}")
                    for ko in range(NKO)
                ]
                for ko in range(NKO):
                    nc.sync.dma_start(
                        rr[ko][:, :qn],
                        dsrc[ko * P : (ko + 1) * P, q0 : q0 + qn],
                    )
                ps = pM.tile([P, 480], F32, tag="mps")
                for ko in range(NKO):
                    nc.tensor.matmul(
                        ps[:, :qn],
                        w_inT_r[:, ko, :],
                        rr[ko][:, :qn],
                        start=(ko == 0),
                        stop=(ko == NKO - 1),
                    )
                nc.vector.tensor_scalar_add(dst[:, q0 : q0 + qn], ps[:, :qn], b_in[:])
            if ci == 1:
                late_w["gwv1T"] = load_w(d_gwv1T, [P, P], "gwv1T_r", BF16)
                late_w["A1"] = load_w(d_A1, [P, P], "A1_r")
            elif ci == 3:
                late_w["w1T"] = load_w(d_w1T, [P, 9, P], "w1T_r", BF16)
                late_w["bn1s"] = load_w(d_bn1s, [P, 1], "bn1s", F32)
                late_w["bn1b"] = load_w(d_bn1b, [P, 1], "bn1b", F32)
        gwv1T_r, A1_r = late_w["gwv1T"], late_w["A1"]
        w1T_r, bn1s, bn1b = late_w["w1T"], late_w["bn1s"], late_w["bn1b"]
        A2_r = load_w(d_A2, [P, P], "A2_r")
        gwv2T_r = load_w(d_gwv2T, [P, P], "gwv2T_r", BF16)
        w2T_r = load_w(d_w2T, [P, 9, P], "w2T_r", BF16)
        bn2s = load_w(d_bn2s, [P, 1], "bn2s", F32)
        bn2b = load_w(d_bn2b, [P, 1], "bn2b", F32)
        predT_r = load_w(d_predT, [P, P], "predT_r")

        # ---------- helpers ----------
        def project(lhs_r, src_r, dst_tag):
            # qh = A @ h, written bf16 (energy moving operand)
            dst = act.tile([P, N], BF16, tag=dst_tag)
            for q0, qn in QCH:
                ps = pM.tile([P, 480], F32, tag="mps")
                nc.tensor.matmul(
                    ps[:, :qn], lhs_r[:], src_r[:, q0 : q0 + qn], start=True, stop=True
                )
                nc.vector.tensor_copy(dst[:, q0 : q0 + qn], ps[:, :qn])
            return dst

        def build_vT(gwvT_r, vT_tag):
            # vT[mo][m, c] = sum_c' yp[c', mo*P+m] * (gamma*wv^T)[c', c]
            # = one matmul per m-chunk with yp as stationary: no transposes
            vT = act.tile([P, NMO, P], BF16, tag=vT_tag)
            for mo in range(NMO):
                pst = pM.tile([P, 480], F32, tag="mps")
                nc.tensor.matmul(
                    pst[:, :P], yp[:, mo * P : (mo + 1) * P], gwvT_r[:],
                    start=True, stop=True,
                )
                nc.vector.tensor_copy(vT[:, mo, :], pst[:, :P])
            return vT

        def zero_pad_border(pad):
            nc.vector.tensor_copy(pad[:, 0, :], zrow[:])
            nc.vector.tensor_copy(pad[:, H + 1, :], zrow[:])
            nc.vector.tensor_copy(pad[:, 1 : H + 1, 0:1], zrow[:, :H, None])
            nc.vector.tensor_copy(pad[:, 1 : H + 1, W + 1 : W + 2], zrow[:, :H, None])

        class ConvStats:
            """BN-stat accumulators + per-rowchunk conv emission."""

            def __init__(self, wT_r, idx):
                self.wT_r = wT_r
                self.idx = idx
                self.t_sb = act.tile([P, N], F32, tag="tagT", name=f"tsb{idx}")
                self.sums = ew.tile([P, len(ROWCH)], F32, tag="sums", name=f"sums{idx}")
                self.sqs = ew.tile([P, len(ROWCH)], F32, tag="sqs", name=f"sqs{idx}")
                self.ps = None

            def tap(self, pad, ci, t):
                # one shifted-window matmul of conv rowchunk ci
                r0, nr = ROWCH[ci]
                qn = nr * W
                if t == 0:
                    self.ps = pM.tile(
                        [P, 480], F32, tag="mps", name=f"cps{self.idx}_{ci}"
                    )
                dy, dx = t // 3, t % 3
                nc.tensor.matmul(
                    self.ps[:, :qn],
                    self.wT_r[:, t, :],
                    pad[:, dy + r0 : dy + r0 + nr, dx : dx + W],
                    start=(t == 0),
                    stop=(t == 8),
                )

            def epilogue(self, ci):
                # drain psum: pre-BN copy + batch-stat partials (all DVE)
                r0, nr = ROWCH[ci]
                qn = nr * W
                q0 = r0 * W
                ps = self.ps
                nc.vector.tensor_copy(self.t_sb[:, q0 : q0 + qn], ps[:, :qn])
                nc.vector.reduce_sum(self.sums[:, ci : ci + 1], ps[:, :qn], axis=AX.X)
                scr = ew.tile([P, 480], F32, tag="sq_scr")
                nc.vector.affine_mul_reduce(
                    scr[:, :qn], self.sqs[:, ci : ci + 1],
                    ps[:, :qn], self.t_sb[:, q0 : q0 + qn],
                    1.0, 0.0,
                )

            def chunk(self, pad, ci):
                for t in range(9):
                    self.tap(pad, ci, t)
                self.epilogue(ci)

        def attention(qh_r, vT_r, resid_r, pad_tag, conv=None):
            # conv: ConvStats for the conv3x3 that CONSUMES this
            # attention's output — rowchunk k-2's taps are interleaved
            # one-per-pair-step into superblock k (its pad rows are
            # complete once superblock k-1 is done).
            pad = act.tile([P, H + 2, W + 2], BF16, tag=pad_tag, name=f"pad_{pad_tag}")
            zero_pad_border(pad)
            NPAIR = NMO // 2  # 9
            steps = [(qi, j) for qi in range(len(QCH)) for j in range(NPAIR)]

            def emit_energy(qi, j):
                q0, qn = QCH[qi]
                mo0, mo1 = 2 * j, 2 * j + 1
                ps_e = pE.tile([P, 1024], F32, tag="energy")
                nc.tensor.matmul(
                    ps_e[:, :qn],
                    yp[:, mo0 * P : (mo0 + 1) * P],
                    qh_r[:, q0 : q0 + qn],
                    start=True, stop=True,
                )
                nc.tensor.matmul(
                    ps_e[:, 512 : 512 + qn],
                    yp[:, mo1 * P : (mo1 + 1) * P],
                    qh_r[:, q0 : q0 + qn],
                    start=True, stop=True,
                )
                return ps_e

            # software pipeline: energies one pair ahead of exp/out
            pend = emit_energy(*steps[0])
            ps_o = ps_s = None
            for idx, (qi, j) in enumerate(steps):
                q0, qn = QCH[qi]
                mo0, mo1 = 2 * j, 2 * j + 1
                ps_e = pend
                pend = emit_energy(*steps[idx + 1]) if idx + 1 < len(steps) else None
                if j == 0:
                    ps_o = pO.tile([P, 480], F32, tag="attn_out")
                    ps_s = pS.tile([P, 480], F32, tag="colsum")
                    prs = []
                # one strided Exp covers both m-chunks of the pair
                ee = eeP.tile([P, 2, 480], BF16, tag="ee")
                nc.scalar.activation(
                    ee[:, :, :qn],
                    ps_e[:, : 2 * 512].rearrange("p (a b) -> p a b", b=512)[:, :, :qn],
                    AF.Exp,
                )
                nc.tensor.matmul(
                    ps_o[:, :qn], vT_r[:, mo0, :], ee[:, 0, :qn],
                    start=(j == 0), stop=False,
                )
                nc.tensor.matmul(
                    ps_o[:, :qn], vT_r[:, mo1, :], ee[:, 1, :qn],
                    start=False, stop=(j == NPAIR - 1),
                )
                # conv taps ride one pair late so tap 0 never waits on the
                # previous superblock's pad-write DVE chain
                if conv is not None and qi >= 2 and j >= 1:
                    conv.tap(pad, qi - 2, j - 1)
                    if j == NPAIR - 1:
                        conv.tap(pad, qi - 2, 8)
                # pair-sum, folded incrementally (binary counter; <=4 live
                # tiles) on DVE/GPSIMD; one partition-sum matmul per qsb
                pr = eeP.tile([P, 480], BF16, tag="pair")
                eng = nc.gpsimd if j % 3 == 1 else nc.vector
                eng.tensor_tensor(
                    pr[:, :qn], ee[:, 0, :qn], ee[:, 1, :qn], ALU.add
                )
                lv, t = 0, pr
                while prs and prs[-1][0] == lv:
                    prev = prs.pop()[1]
                    o = eeP.tile([P, 480], BF16, tag="fold")
                    e2 = nc.gpsimd if (j + lv) % 3 == 2 else nc.vector
                    e2.tensor_tensor(o[:, :qn], prev[:, :qn], t[:, :qn], ALU.add)
                    t, lv = o, lv + 1
                prs.append((lv, t))
                if j == NPAIR - 1:
                    while len(prs) > 1:
                        (_, a), (_, b2) = prs.pop(), prs.pop()
                        o = eeP.tile([P, 480], BF16, tag="fold")
                        nc.vector.tensor_tensor(
                            o[:, :qn], a[:, :qn], b2[:, :qn], ALU.add
                        )
                        prs.append((99, o))
                    nc.tensor.matmul(
                        ps_s[:, :qn], ones_b[:], prs.pop()[1][:, :qn],
                        start=True, stop=True,
                    )
                    rcp = ew.tile([P, 480], F32, tag="recip")
                    nc.vector.reciprocal_approx_fast(rcp[:, :qn], ps_s[:, :qn])
                    tmp = ew.tile([P, 480], F32, tag="tmp")
                    nc.vector.tensor_tensor(
                        tmp[:, :qn], ps_o[:, :qn], rcp[:, :qn], ALU.mult
                    )
                    r0, nr = q0 // W, qn // W
                    nc.vector.tensor_tensor(
                        pad[:, 1 + r0 : 1 + r0 + nr, 1 : W + 1],
                        tmp[:, :qn].rearrange("p (a b) -> p a b", b=W),
                        resid_r[:, q0 : q0 + qn].rearrange("p (a b) -> p a b", b=W),
                        ALU.add,
                    )
                    if conv is not None and qi >= 2:
                        conv.epilogue(qi - 2)
            return pad

        def bn_relu(conv, pad, bns, bnb, out_tag, ar_idx, overlap_fn=None,
                    tail_fn=None):
            # finish conv rowchunks 3,4, then batch-stat allreduce + BN+ReLU
            for ci in (3, 4):
                conv.chunk(pad, ci)
            stats = ew.tile([P, 2], F32, tag="stats")
            nc.vector.reduce_sum(stats[:, 0:1], conv.sums[:], axis=AX.X)
            nc.vector.reduce_sum(stats[:, 1:2], conv.sqs[:], axis=AX.X)
            nc.vector.tensor_scalar_mul(stats[:], stats[:], 1.0 / NSTAT)
            cc_in = dram.tile([P, 2], F32, tag=f"cc_in{ar_idx}")
            cc_out = dram.tile([P, 2], F32, tag=f"cc_out{ar_idx}")
            nc.sync.dma_start(cc_in[:], stats[:])
            if sim:
                nc.sync.dma_start(cc_out[:], cc_in[:])
            else:
                nc.gpsimd.collective_compute(
                    "AllReduce",
                    ALU.add,
                    replica_groups=[list(range(8))],
                    ins=[cc_in[:].opt()],
                    outs=[cc_out[:].opt()],
                )
            if overlap_fn is not None:
                overlap_fn()
            st_all = ew.tile([P, 2], F32, tag="st_all")
            nc.sync.dma_start(st_all[:], cc_out[:])
            mean = st_all[:, 0:1]
            var = ew.tile([P, 1], F32, tag="var")
            nc.vector.tensor_tensor(var[:], mean, mean, ALU.mult)
            # var = m2 - mean^2 + eps, fused: (var * -1 + m2) then +eps
            nc.vector.scalar_tensor_tensor(
                var[:], var[:], -1.0, st_all[:, 1:2], ALU.mult, ALU.add
            )
            nc.vector.tensor_scalar_add(var[:], var[:], EPS)
            std = ew.tile([P, 1], F32, tag="std")
            nc.scalar.activation(std[:], var[:], AF.Sqrt)
            a_sc = ew.tile([P, 1], F32, tag="a_sc")
            with nc.allow_low_precision(reason="bn rsqrt"):
                nc.vector.reciprocal(a_sc[:], std[:])
            nc.vector.tensor_tensor(a_sc[:], a_sc[:], bns[:], ALU.mult)
            c_bi = ew.tile([P, 1], F32, tag="c_bi")
            # c = bnb - mean*a
            nc.vector.tensor_tensor(c_bi[:], mean, a_sc[:], ALU.mult)
            nc.vector.tensor_tensor(c_bi[:], bnb[:], c_bi[:], ALU.subtract)
            h_out = act.tile([P, N], F32R, tag=out_tag, name=f"h_{ar_idx}")
            for q0, qn in QCH:
                nc.scalar.activation(
                    h_out[:, q0 : q0 + qn], conv.t_sb[:, q0 : q0 + qn],
                    AF.Relu, bias=c_bi[:], scale=a_sc[:],
                )
                if tail_fn is not None:
                    tail_fn(h_out, q0, qn)
            return h_out

        # ---------- pipeline ----------
        vT1 = build_vT(gwv1T_r, "vT1")
        qh1 = project(A1_r, xp, "tagC")
        conv1 = ConvStats(w1T_r, 1)
        h1pad = attention(qh1, vT1, xp, "tagE", conv=conv1)

        # bn1's tail interleaves the qh2 projection with the per-chunk ReLUs
        qh2 = act.tile([P, N], BF16, tag="tagC", name="qh2")

        def qh2_tail(h_out, q0, qn):
            ps = pM.tile([P, 480], F32, tag="mps", name=f"qh2ps{q0}")
            nc.tensor.matmul(
                ps[:, :qn], A2_r[:], h_out[:, q0 : q0 + qn], start=True, stop=True
            )
            nc.vector.tensor_copy(qh2[:, q0 : q0 + qn], ps[:, :qn])

        vT2_box = []
        h2 = bn_relu(
            conv1, h1pad, bn1s, bn1b, "h2", 1,
            overlap_fn=lambda: vT2_box.append(build_vT(gwv2T_r, "vT2")),
            tail_fn=qh2_tail,
        )
        vT2 = vT2_box[0]
        conv2 = ConvStats(w2T_r, 2)
        h3pad = attention(qh2, vT2, h2, "tagE", conv=conv2)

        # bn2's tail interleaves the pred head (pred_b added host-side)
        out_sb = act.tile([1, N], F32, tag="out_sb")

        def pred_tail(h_out, q0, qn):
            ps = pM.tile([P, 480], F32, tag="mps", name=f"predps{q0}")
            nc.tensor.matmul(
                ps[:, :qn], predT_r[:], h_out[:, q0 : q0 + qn], start=True, stop=True
            )
            nc.vector.tensor_copy(out_sb[:, q0 : q0 + qn], ps[0:1, :qn])

        bn_relu(conv2, h3pad, bn2s, bn2b, "h2", 2, tail_fn=pred_tail)
        nc.sync.dma_start(d_out[:, :], out_sb[:])

    nc.compile()
    _NC_CACHE[key] = nc
    return nc


def _install_ntff_hook():
    """Register the axon NTFF profiling hook (antenv.axon_hooks is absent
    in this image; libaxon_pjrt.so exports the C ABI — same wiring as
    trn_agent_boot's _ntff_profile_via_ctypes)."""
    import sys as _sys, types, ctypes, contextlib

    if "antenv.axon_hooks" in _sys.modules:
        return
    try:
        lib = ctypes.CDLL("/opt/axon/libaxon_pjrt.so")
        lib.axon_start_nrt_profile.argtypes = [
            ctypes.POINTER(ctypes.c_int64), ctypes.c_size_t,
        ]
        lib.axon_start_nrt_profile.restype = ctypes.c_int64
        lib.axon_stop_nrt_profile.argtypes = [ctypes.c_char_p]
        lib.axon_stop_nrt_profile.restype = ctypes.c_int64
    except (OSError, AttributeError):
        return

    @contextlib.contextmanager
    def _hook(output_dir, device_ids):
        import jax

        jax.devices()
        if device_ids:
            ids = (ctypes.c_int64 * len(device_ids))(*device_ids)
            rc = lib.axon_start_nrt_profile(ids, len(device_ids))
        else:
            rc = lib.axon_start_nrt_profile(None, 0)
        if rc != 0:
            raise RuntimeError(f"axon_start_nrt_profile rc={rc}")
        try:
            yield
        finally:
            n = lib.axon_stop_nrt_profile(str(output_dir).encode())
            if n < 0:
                raise RuntimeError(f"axon_stop_nrt_profile rc={n}")

    mod = types.ModuleType("antenv.axon_hooks")
    mod.get_axon_ntff_profile_hook = lambda: _hook
    mod.set_axon_ntff_profile_hook = lambda h: None
    _sys.modules["antenv.axon_hooks"] = mod
    # artifact upload has no bucket in this container; keep files local
    import concourse.bass_utils as _bu

    _bu.upload_artifacts = lambda d: d


def kernel(**inputs):
    from concourse.bass_utils import run_bass_kernel_spmd
    import ml_dtypes
    import os

    nc = _build()

    f32 = np.float32
    bf16 = ml_dtypes.bfloat16
    x = np.asarray(inputs["x"], dtype=f32).reshape(B, CIN, N).astype(bf16)
    y = np.asarray(inputs["y"], dtype=f32).reshape(B, CIN, N).astype(bf16)
    w_in = np.asarray(inputs["w_in"], dtype=f32)
    b_in = np.asarray(inputs["b_in"], dtype=f32).reshape(P, 1)
    ca_wq = np.asarray(inputs["ca_wq"], dtype=f32)
    ca_wk = np.asarray(inputs["ca_wk"], dtype=f32)
    ca_wv = np.asarray(inputs["ca_wv"], dtype=f32)
    g1 = np.asarray(inputs["ca_gamma"], dtype=f32).reshape(-1)[0]
    sa_wq = np.asarray(inputs["sa_wq"], dtype=f32)
    sa_wk = np.asarray(inputs["sa_wk"], dtype=f32)
    sa_wv = np.asarray(inputs["sa_wv"], dtype=f32)
    g2 = np.asarray(inputs["sa_gamma"], dtype=f32).reshape(-1)[0]
    conv1_w = np.asarray(inputs["conv1_w"], dtype=f32)
    conv2_w = np.asarray(inputs["conv2_w"], dtype=f32)
    bn1s = np.asarray(inputs["bn1_s"], dtype=f32).reshape(P, 1)
    bn1b = np.asarray(inputs["bn1_b"], dtype=f32).reshape(P, 1)
    bn2s = np.asarray(inputs["bn2_s"], dtype=f32).reshape(P, 1)
    bn2b = np.asarray(inputs["bn2_b"], dtype=f32).reshape(P, 1)
    pred_w = np.asarray(inputs["pred_w"], dtype=f32)
    pred_b = np.asarray(inputs["pred_b"], dtype=f32).reshape(1, 1)

    # host-side weight prep (small, O(C^2))
    w_inT = np.ascontiguousarray(
        w_in.T.reshape(NKO, P, P).transpose(1, 0, 2)
    ).astype(bf16)  # [cin_p, ko, cout]
    A1 = np.ascontiguousarray(ca_wq.T @ ca_wk)
    A2 = np.ascontiguousarray(sa_wq.T @ sa_wk)
    # conv taps: [o, i, 3, 3] -> lhsT per tap [i, o]; layout [i_p, tap, o]
    w1T = np.ascontiguousarray(
        conv1_w.transpose(2, 3, 1, 0).reshape(9, P, P).transpose(1, 0, 2)
    ).astype(bf16)
    w2T = np.ascontiguousarray(
        conv2_w.transpose(2, 3, 1, 0).reshape(9, P, P).transpose(1, 0, 2)
    ).astype(bf16)
    predT = np.zeros((P, P), f32)
    predT[:, 0] = pred_w[0]

    shared = {
        "w_inT": w_inT, "b_in": b_in, "A1": A1,
        "gwv1T": np.ascontiguousarray(g1 * ca_wv.T).astype(bf16),
        "A2": A2, "gwv2T": np.ascontiguousarray(g2 * sa_wv.T).astype(bf16),
        "w1T": w1T, "bn1s": bn1s,
        "bn1b": bn1b, "w2T": w2T, "bn2s": bn2s, "bn2b": bn2b,
        "predT": predT,
    }
    in_maps = [
        {"x": np.ascontiguousarray(x[i]), "y": np.ascontiguousarray(y[i]), **shared}
        for i in range(B)
    ]

    trace = bool(int(os.environ.get("KERNEL_TRACE", "0")))
    if trace:
        _install_ntff_hook()
    res = run_bass_kernel_spmd(nc, in_maps, core_ids=list(range(B)), trace=trace)
    if trace:
        _NC_CACHE["last_results"] = res
    out = np.stack(
        [res.results[i]["out"].reshape(1, H, W) for i in range(B)]
    ).astype(f32)
    return out + pred_b[0, 0]


# revision 28
# speedup vs baseline: 1.2375x; 1.2375x over previous
"""Trainium2 Bass kernel for nn_CrossAttention (dense_transformer).

Sharding: data-parallel over batch B=8 across 8 NeuronCores (1 sample
per core). BatchNorm uses batch statistics, so per-channel partial
[sum, sumsq] are all-reduced across cores ([128,2] f32 payload, 2x).

Per-core layout: activations [C=128 partitions, N=H*W=2304 free].
Matmul dtype strategy: everything except the qh projections / pred
head runs bf16 x bf16 (inputs are cast to bf16 on the host, halving
the input DMA stream that dominates startup and cross-core skew; yp,
qh, pad, conv weights, v^T, exp'd energies are bf16 on device). bf16
stationaries enable the PE's fast-weight-load path (~100ns vs ~220ns
f32r per 128-col LDWEIGHTS) and 1 cycle/row at any free size. PSUM
accumulation is fp32 throughout; the xp/h2 residual paths stay fp32.

Attention in energy-transposed layout with algebraically folded
projections (saves the k-projection and all per-block transposes):
  energy^T[m,q] = sum_c yp[c,m] * qh[c,q],  qh = (Wq^T Wk)^T @ h
  v^T[mo]       = matmul(lhsT=yp[:,mo-chunk], rhs=gamma*Wv^T)
Softmax (no max-subtraction needed: |energy| <= ~10) reduces over m =
partitions: exp'd energies are pair-summed (two m-chunks share one
2-bank PSUM tile so each Exp activation covers both via a strided AP),
tree-folded on DVE/GPSIMD, and one ones-matmul per q-superblock does
the partition sum, arriving pre-broadcast across partitions; 1/colsum
is applied after the v-contraction (reciprocal_approx_fast).

conv3x3 = 9 shifted-window matmuls over a zero-padded [128,50,50]
bf16 buffer. The attention inner loop is Scalar(Exp)-bound (~1.03us
per pair-step vs ~0.82us of matmul), so conv rowchunk k-2's taps are
interleaved one-per-pair-step into attention superblock k — the conv
rides in the PE bubbles and only rowchunks 3,4 remain after the loop.
BN sumsq runs on DVE (affine_mul_reduce), keeping Scalar for Exp.

A warmup AllReduce (unread result) fires at kernel start: it absorbs
the cross-core launch skew and the CC-core cold-start. pred_b is
folded into the host-side gather.
"""

import sys

sys.path.insert(0, "/opt/trn_rl_repo")

import numpy as np

_NC_CACHE = {}

B, CIN, C, H, W = 8, 256, 128, 48, 48
N = H * W  # 2304
P = 128
NKO = CIN // P  # 2
NMO = N // P  # 18
# q superblocks: row-aligned chunks (48-col rows); 480 = 10 rows
QCH = [(0, 480), (480, 480), (960, 480), (1440, 480), (1920, 384)]
ROWCH = [(0, 10), (10, 10), (20, 10), (30, 10), (40, 8)]
# stage-A load/projection chunks: finer at the start so the first
# matmul fires early in the input DMA stream
LCH = [(0, 256), (256, 256), (512, 448), (960, 480), (1440, 480), (1920, 384)]
NSTAT = float(B * N)  # BN stat count over (B,H,W)
EPS = 1e-5


def _build(variant="default"):
    """variant: 'default' = 8-core w/ collectives; 'sim' = single-core,
    collectives replaced by DMA copy (for TimelineSim profiling)."""
    key = f"nc_{variant}"
    if key in _NC_CACHE:
        return _NC_CACHE[key]

    import concourse.mybir as mybir
    import concourse.tile as tile
    from concourse import bacc
    from contextlib import ExitStack

    F32 = mybir.dt.float32
    F32R = mybir.dt.float32r
    BF16 = mybir.dt.bfloat16
    AF = mybir.ActivationFunctionType
    ALU = mybir.AluOpType
    AX = mybir.AxisListType

    sim = variant == "sim"
    nc = bacc.Bacc(
        "TRN2", target_bir_lowering=False, debug=False,
        num_devices=1 if sim else 8,
    )

    # ---- DRAM I/O ----
    d_x = nc.dram_tensor("x", [CIN, N], BF16, kind="ExternalInput")
    d_y = nc.dram_tensor("y", [CIN, N], BF16, kind="ExternalInput")
    d_w_inT = nc.dram_tensor("w_inT", [P, NKO, P], BF16, kind="ExternalInput")
    d_b_in = nc.dram_tensor("b_in", [P, 1], F32, kind="ExternalInput")
    d_A1 = nc.dram_tensor("A1", [P, P], F32R, kind="ExternalInput")
    d_gwv1T = nc.dram_tensor("gwv1T", [P, P], BF16, kind="ExternalInput")
    d_A2 = nc.dram_tensor("A2", [P, P], F32R, kind="ExternalInput")
    d_gwv2T = nc.dram_tensor("gwv2T", [P, P], BF16, kind="ExternalInput")
    d_w1T = nc.dram_tensor("w1T", [P, 9, P], BF16, kind="ExternalInput")
    d_bn1s = nc.dram_tensor("bn1s", [P, 1], F32, kind="ExternalInput")
    d_bn1b = nc.dram_tensor("bn1b", [P, 1], F32, kind="ExternalInput")
    d_w2T = nc.dram_tensor("w2T", [P, 9, P], BF16, kind="ExternalInput")
    d_bn2s = nc.dram_tensor("bn2s", [P, 1], F32, kind="ExternalInput")
    d_bn2b = nc.dram_tensor("bn2b", [P, 1], F32, kind="ExternalInput")
    d_predT = nc.dram_tensor("predT", [P, P], F32R, kind="ExternalInput")
    d_out = nc.dram_tensor("out", [1, N], F32, kind="ExternalOutput")

    with tile.TileContext(nc) as tc, ExitStack() as ctx:
        wgt = ctx.enter_context(tc.tile_pool(name="wgt", bufs=1))
        act = ctx.enter_context(tc.tile_pool(name="act", bufs=1))
        ew = ctx.enter_context(tc.tile_pool(name="ew", bufs=1))
        eeP = ctx.enter_context(tc.tile_pool(name="eeP", bufs=6))
        load = ctx.enter_context(tc.tile_pool(name="load", bufs=12))
        dram = ctx.enter_context(tc.tile_pool(name="dram", bufs=1, space="DRAM"))
        pE = ctx.enter_context(tc.tile_pool(name="pE", bufs=2, space="PSUM"))
        pO = ctx.enter_context(tc.tile_pool(name="pO", bufs=1, space="PSUM"))
        pS = ctx.enter_context(tc.tile_pool(name="pS", bufs=1, space="PSUM"))
        pM = ctx.enter_context(tc.tile_pool(name="pM", bufs=2, space="PSUM"))

        # ---------- warmup AllReduce: unread result; absorbs launch skew
        # and CC-core cold start while the input DMAs stream ----------
        if not sim:
            cc_w_in = dram.tile([P, 2], F32, tag="cc_w_in")
            cc_w_out = dram.tile([P, 2], F32, tag="cc_w_out")
            nc.gpsimd.collective_compute(
                "AllReduce",
                mybir.AluOpType.add,
                replica_groups=[list(range(8))],
                ins=[cc_w_in[:].opt()],
                outs=[cc_w_out[:].opt()],
            )

        # ---------- weights (direct DMA) ----------
        def load_w(dsrc, shape, tag, dtype=F32R):
            t = wgt.tile(shape, dtype, tag=tag)
            nc.sync.dma_start(t[:], dsrc[...])
            return t

        # DMA emission order controls queue priority: only the weights
        # needed for the input projections go ahead of the x/y stream;
        # everything else is enqueued between/after the input chunks.
        w_inT_r = load_w(d_w_inT, [P, NKO, P], "w_inT_r", BF16)
        b_in = load_w(d_b_in, [P, 1], "b_in", F32)

        ones_b = wgt.tile([P, P], BF16, tag="ones_b")
        nc.gpsimd.memset(ones_b[:], 1.0)

        zrow = wgt.tile([P, W + 2], BF16, tag="zrow")
        nc.gpsimd.memset(zrow[:], 0.0)

        # ---------- stage A (feeder-driven): x/y chunk loads, input
        # projections, vT1 build and qh1 projection are emitted on demand
        # by the attention loop, so attention superblock 0 starts as soon
        # as the first chunks land while the DMA tail streams underneath.
        # yp is written bf16 (energy/vT stationary); xp stays f32r
        # (qh1 moving operand + attn1 residual).
        xp = act.tile([P, N], F32R, tag="tagA")
        yp = act.tile([P, N], BF16, tag="tagB")
        late_w = {}

        class StageFeeder:
            def __init__(self, vT1, qh1):
                self.ci = 0
                self.cov = 0
                self.vmo = 0
                self.qqi = 0
                self.vT1 = vT1
                self.qh1 = qh1

            def _chunk(self):
                ci = self.ci
                q0, qn = LCH[ci]
                for ti, (dsrc, dst) in enumerate(((d_x, xp), (d_y, yp))):
                    rr = [
                        load.tile([P, 480], BF16, tag="in_r", name=f"ld{ci}_{ti}_{ko}")
                        for ko in range(NKO)
                    ]
                    for ko in range(NKO):
                        nc.sync.dma_start(
                            rr[ko][:, :qn],
                            dsrc[ko * P : (ko + 1) * P, q0 : q0 + qn],
                        )
                    ps = pM.tile([P, 480], F32, tag="mps", name=f"aps{ci}_{ti}")
                    for ko in range(NKO):
                        nc.tensor.matmul(
                            ps[:, :qn],
                            w_inT_r[:, ko, :],
                            rr[ko][:, :qn],
                            start=(ko == 0),
                            stop=(ko == NKO - 1),
                        )
                    nc.vector.tensor_scalar_add(
                        dst[:, q0 : q0 + qn], ps[:, :qn], b_in[:]
                    )
                if ci == 0:
                    late_w["gwv1T"] = load_w(d_gwv1T, [P, P], "gwv1T_r", BF16)
                    late_w["A1"] = load_w(d_A1, [P, P], "A1_r")
                elif ci == 3:
                    late_w["w1T"] = load_w(d_w1T, [P, 9, P], "w1T_r", BF16)
                    late_w["bn1s"] = load_w(d_bn1s, [P, 1], "bn1s", F32)
                    late_w["bn1b"] = load_w(d_bn1b, [P, 1], "bn1b", F32)
                self.ci += 1
                self.cov = q0 + qn

            def _vT(self):
                mo = self.vmo
                pst = pM.tile([P, 480], F32, tag="mps", name=f"vps{mo}")
                nc.tensor.matmul(
                    pst[:, :P], yp[:, mo * P : (mo + 1) * P],
                    late_w["gwv1T"][:], start=True, stop=True,
                )
                nc.vector.tensor_copy(self.vT1[:, mo, :], pst[:, :P])
                self.vmo += 1

            def _qh(self):
                qi = self.qqi
                q0, qn = QCH[qi]
                ps = pM.tile([P, 480], F32, tag="mps", name=f"qps{qi}")
                nc.tensor.matmul(
                    ps[:, :qn], late_w["A1"][:], xp[:, q0 : q0 + qn],
                    start=True, stop=True,
                )
                nc.vector.tensor_copy(self.qh1[:, q0 : q0 + qn], ps[:, :qn])
                self.qqi += 1

            def ensure(self, qi, j):
                need = (2 * j + 2) * P
                while self.cov < need:
                    self._chunk()
                while self.vmo < 2 * j + 2:
                    self._vT()
                while self.qqi <= qi:
                    end = QCH[self.qqi][0] + QCH[self.qqi][1]
                    while self.cov < end:
                        self._chunk()
                    self._qh()

        # ---------- helpers ----------
        def build_vT(gwvT_r, vT_tag):
            # vT[mo][m, c] = sum_c' yp[c', mo*P+m] * (gamma*wv^T)[c', c]
            # = one matmul per m-chunk with yp as stationary: no transposes
            vT = act.tile([P, NMO, P], BF16, tag=vT_tag)
            for mo in range(NMO):
                pst = pM.tile([P, 480], F32, tag="mps")
                nc.tensor.matmul(
                    pst[:, :P], yp[:, mo * P : (mo + 1) * P], gwvT_r[:],
                    start=True, stop=True,
                )
                nc.vector.tensor_copy(vT[:, mo, :], pst[:, :P])
            return vT

        def zero_pad_border(pad):
            nc.vector.tensor_copy(pad[:, 0, :], zrow[:])
            nc.vector.tensor_copy(pad[:, H + 1, :], zrow[:])
            nc.vector.tensor_copy(pad[:, 1 : H + 1, 0:1], zrow[:, :H, None])
            nc.vector.tensor_copy(pad[:, 1 : H + 1, W + 1 : W + 2], zrow[:, :H, None])

        class ConvStats:
            """BN-stat accumulators + per-rowchunk conv emission."""

            def __init__(self, wT_get, idx):
                self.wT_get = wT_get
                self.idx = idx
                self.t_sb = act.tile([P, N], F32, tag="tagT", name=f"tsb{idx}")
                self.sums = ew.tile([P, len(ROWCH)], F32, tag="sums", name=f"sums{idx}")
                self.sqs = ew.tile([P, len(ROWCH)], F32, tag="sqs", name=f"sqs{idx}")
                self.ps = None

            def tap(self, pad, ci, t):
                # one shifted-window matmul of conv rowchunk ci
                r0, nr = ROWCH[ci]
                qn = nr * W
                if t == 0:
                    self.ps = pM.tile(
                        [P, 480], F32, tag="mps", name=f"cps{self.idx}_{ci}"
                    )
                dy, dx = t // 3, t % 3
                nc.tensor.matmul(
                    self.ps[:, :qn],
                    self.wT_get()[:, t, :],
                    pad[:, dy + r0 : dy + r0 + nr, dx : dx + W],
                    start=(t == 0),
                    stop=(t == 8),
                )

            def epilogue(self, ci):
                # drain psum: pre-BN copy + batch-stat partials (all DVE)
                r0, nr = ROWCH[ci]
                qn = nr * W
                q0 = r0 * W
                ps = self.ps
                nc.vector.tensor_copy(self.t_sb[:, q0 : q0 + qn], ps[:, :qn])
                nc.vector.reduce_sum(self.sums[:, ci : ci + 1], ps[:, :qn], axis=AX.X)
                scr = ew.tile([P, 480], F32, tag="sq_scr")
                nc.vector.affine_mul_reduce(
                    scr[:, :qn], self.sqs[:, ci : ci + 1],
                    ps[:, :qn], self.t_sb[:, q0 : q0 + qn],
                    1.0, 0.0,
                )

            def chunk(self, pad, ci):
                for t in range(9):
                    self.tap(pad, ci, t)
                self.epilogue(ci)

        def attention(qh_r, vT_r, resid_r, pad_tag, conv=None, feeder=None):
            # conv: ConvStats for the conv3x3 that CONSUMES this
            # attention's output — rowchunk k-2's taps are interleaved
            # one-per-pair-step into superblock k (its pad rows are
            # complete once superblock k-1 is done).
            pad = act.tile([P, H + 2, W + 2], BF16, tag=pad_tag, name=f"pad_{pad_tag}")
            zero_pad_border(pad)
            NPAIR = NMO // 2  # 9
            steps = [(qi, j) for qi in range(len(QCH)) for j in range(NPAIR)]

            def emit_energy(qi, j):
                if feeder is not None:
                    feeder.ensure(qi, j)
                q0, qn = QCH[qi]
                mo0, mo1 = 2 * j, 2 * j + 1
                ps_e = pE.tile([P, 1024], F32, tag="energy")
                nc.tensor.matmul(
                    ps_e[:, :qn],
                    yp[:, mo0 * P : (mo0 + 1) * P],
                    qh_r[:, q0 : q0 + qn],
                    start=True, stop=True,
                )
                nc.tensor.matmul(
                    ps_e[:, 512 : 512 + qn],
                    yp[:, mo1 * P : (mo1 + 1) * P],
                    qh_r[:, q0 : q0 + qn],
                    start=True, stop=True,
                )
                return ps_e

            # software pipeline: energies one pair ahead of exp/out
            pend = emit_energy(*steps[0])
            ps_o = ps_s = None
            for idx, (qi, j) in enumerate(steps):
                q0, qn = QCH[qi]
                mo0, mo1 = 2 * j, 2 * j + 1
                ps_e = pend
                pend = emit_energy(*steps[idx + 1]) if idx + 1 < len(steps) else None
                if j == 0:
                    ps_o = pO.tile([P, 480], F32, tag="attn_out")
                    ps_s = pS.tile([P, 480], F32, tag="colsum")
                    prs = []
                # one strided Exp covers both m-chunks of the pair
                ee = eeP.tile([P, 2, 480], BF16, tag="ee")
                nc.scalar.activation(
                    ee[:, :, :qn],
                    ps_e[:, : 2 * 512].rearrange("p (a b) -> p a b", b=512)[:, :, :qn],
                    AF.Exp,
                )
                nc.tensor.matmul(
                    ps_o[:, :qn], vT_r[:, mo0, :], ee[:, 0, :qn],
                    start=(j == 0), stop=False,
                )
                nc.tensor.matmul(
                    ps_o[:, :qn], vT_r[:, mo1, :], ee[:, 1, :qn],
                    start=False, stop=(j == NPAIR - 1),
                )
                if conv is not None and qi >= 2:
                    conv.tap(pad, qi - 2, j)
                # pair-sum, folded incrementally (binary counter; <=4 live
                # tiles) on DVE/GPSIMD; one partition-sum matmul per qsb
                pr = eeP.tile([P, 480], BF16, tag="pair")
                eng = nc.gpsimd if j % 3 == 1 else nc.vector
                eng.tensor_tensor(
                    pr[:, :qn], ee[:, 0, :qn], ee[:, 1, :qn], ALU.add
                )
                lv, t = 0, pr
                while prs and prs[-1][0] == lv:
                    prev = prs.pop()[1]
                    o = eeP.tile([P, 480], BF16, tag="fold")
                    e2 = nc.gpsimd if (j + lv) % 3 == 2 else nc.vector
                    e2.tensor_tensor(o[:, :qn], prev[:, :qn], t[:, :qn], ALU.add)
                    t, lv = o, lv + 1
                prs.append((lv, t))
                if j == NPAIR - 1:
                    while len(prs) > 1:
                        (_, a), (_, b2) = prs.pop(), prs.pop()
                        o = eeP.tile([P, 480], BF16, tag="fold")
                        nc.vector.tensor_tensor(
                            o[:, :qn], a[:, :qn], b2[:, :qn], ALU.add
                        )
                        prs.append((99, o))
                    nc.tensor.matmul(
                        ps_s[:, :qn], ones_b[:], prs.pop()[1][:, :qn],
                        start=True, stop=True,
                    )
                    rcp = ew.tile([P, 480], F32, tag="recip")
                    nc.vector.reciprocal_approx_fast(rcp[:, :qn], ps_s[:, :qn])
                    tmp = ew.tile([P, 480], F32, tag="tmp")
                    nc.vector.tensor_tensor(
                        tmp[:, :qn], ps_o[:, :qn], rcp[:, :qn], ALU.mult
                    )
                    r0, nr = q0 // W, qn // W
                    nc.vector.tensor_tensor(
                        pad[:, 1 + r0 : 1 + r0 + nr, 1 : W + 1],
                        tmp[:, :qn].rearrange("p (a b) -> p a b", b=W),
                        resid_r[:, q0 : q0 + qn].rearrange("p (a b) -> p a b", b=W),
                        ALU.add,
                    )
                    if conv is not None and qi >= 2:
                        conv.epilogue(qi - 2)
            return pad

        def bn_relu(conv, pad, bns, bnb, out_tag, ar_idx, overlap_fn=None,
                    tail_fn=None):
            # finish conv rowchunks 3,4, then batch-stat allreduce + BN+ReLU
            for ci in (3, 4):
                conv.chunk(pad, ci)
            stats = ew.tile([P, 2], F32, tag="stats")
            nc.vector.reduce_sum(stats[:, 0:1], conv.sums[:], axis=AX.X)
            nc.vector.reduce_sum(stats[:, 1:2], conv.sqs[:], axis=AX.X)
            nc.vector.tensor_scalar_mul(stats[:], stats[:], 1.0 / NSTAT)
            cc_in = dram.tile([P, 2], F32, tag=f"cc_in{ar_idx}")
            cc_out = dram.tile([P, 2], F32, tag=f"cc_out{ar_idx}")
            nc.sync.dma_start(cc_in[:], stats[:])
            if sim:
                nc.sync.dma_start(cc_out[:], cc_in[:])
            else:
                nc.gpsimd.collective_compute(
                    "AllReduce",
                    ALU.add,
                    replica_groups=[list(range(8))],
                    ins=[cc_in[:].opt()],
                    outs=[cc_out[:].opt()],
                )
            if overlap_fn is not None:
                overlap_fn()
            st_all = ew.tile([P, 2], F32, tag="st_all")
            nc.sync.dma_start(st_all[:], cc_out[:])
            mean = st_all[:, 0:1]
            var = ew.tile([P, 1], F32, tag="var")
            nc.vector.tensor_tensor(var[:], mean, mean, ALU.mult)
            # var = m2 - mean^2 + eps, fused: (var * -1 + m2) then +eps
            nc.vector.scalar_tensor_tensor(
                var[:], var[:], -1.0, st_all[:, 1:2], ALU.mult, ALU.add
            )
            nc.vector.tensor_scalar_add(var[:], var[:], EPS)
            std = ew.tile([P, 1], F32, tag="std")
            nc.scalar.activation(std[:], var[:], AF.Sqrt)
            a_sc = ew.tile([P, 1], F32, tag="a_sc")
            with nc.allow_low_precision(reason="bn rsqrt"):
                nc.vector.reciprocal(a_sc[:], std[:])
            nc.vector.tensor_tensor(a_sc[:], a_sc[:], bns[:], ALU.mult)
            c_bi = ew.tile([P, 1], F32, tag="c_bi")
            # c = bnb - mean*a
            nc.vector.tensor_tensor(c_bi[:], mean, a_sc[:], ALU.mult)
            nc.vector.tensor_tensor(c_bi[:], bnb[:], c_bi[:], ALU.subtract)
            h_out = act.tile([P, N], F32R, tag=out_tag, name=f"h_{ar_idx}")
            for q0, qn in QCH:
                nc.scalar.activation(
                    h_out[:, q0 : q0 + qn], conv.t_sb[:, q0 : q0 + qn],
                    AF.Relu, bias=c_bi[:], scale=a_sc[:],
                )
                if tail_fn is not None:
                    tail_fn(h_out, q0, qn)
            return h_out

        # ---------- pipeline ----------
        vT1 = act.tile([P, NMO, P], BF16, tag="vT1")
        qh1 = act.tile([P, N], BF16, tag="tagC", name="qh1")
        feeder = StageFeeder(vT1, qh1)
        conv1 = ConvStats(lambda: late_w["w1T"], 1)
        h1pad = attention(qh1, vT1, xp, "tagE", conv=conv1, feeder=feeder)

        # stage-2 weights: enqueued after the input stream so they never
        # delay the startup-critical x/y chunks
        A2_r = load_w(d_A2, [P, P], "A2_r")
        gwv2T_r = load_w(d_gwv2T, [P, P], "gwv2T_r", BF16)
        w2T_r = load_w(d_w2T, [P, 9, P], "w2T_r", BF16)
        bn2s = load_w(d_bn2s, [P, 1], "bn2s", F32)
        bn2b = load_w(d_bn2b, [P, 1], "bn2b", F32)
        predT_r = load_w(d_predT, [P, P], "predT_r")
        bn1s, bn1b = late_w["bn1s"], late_w["bn1b"]

        # bn1's tail interleaves the qh2 projection with the per-chunk ReLUs
        qh2 = act.tile([P, N], BF16, tag="tagC", name="qh2")

        def qh2_tail(h_out, q0, qn):
            ps = pM.tile([P, 480], F32, tag="mps", name=f"qh2ps{q0}")
            nc.tensor.matmul(
                ps[:, :qn], A2_r[:], h_out[:, q0 : q0 + qn], start=True, stop=True
            )
            nc.vector.tensor_copy(qh2[:, q0 : q0 + qn], ps[:, :qn])

        vT2_box = []
        h2 = bn_relu(
            conv1, h1pad, bn1s, bn1b, "h2", 1,
            overlap_fn=lambda: vT2_box.append(build_vT(gwv2T_r, "vT2")),
            tail_fn=qh2_tail,
        )
        vT2 = vT2_box[0]
        conv2 = ConvStats(lambda: w2T_r, 2)
        h3pad = attention(qh2, vT2, h2, "tagE", conv=conv2)

        # bn2's tail interleaves the pred head (pred_b added host-side)
        out_sb = act.tile([1, N], F32, tag="out_sb")

        def pred_tail(h_out, q0, qn):
            ps = pM.tile([P, 480], F32, tag="mps", name=f"predps{q0}")
            nc.tensor.matmul(
                ps[:, :qn], predT_r[:], h_out[:, q0 : q0 + qn], start=True, stop=True
            )
            nc.vector.tensor_copy(out_sb[:, q0 : q0 + qn], ps[0:1, :qn])

        bn_relu(conv2, h3pad, bn2s, bn2b, "h2", 2, tail_fn=pred_tail)
        nc.sync.dma_start(d_out[:, :], out_sb[:])

    nc.compile()
    _NC_CACHE[key] = nc
    return nc


def _install_ntff_hook():
    """Register the axon NTFF profiling hook (antenv.axon_hooks is absent
    in this image; libaxon_pjrt.so exports the C ABI — same wiring as
    trn_agent_boot's _ntff_profile_via_ctypes)."""
    import sys as _sys, types, ctypes, contextlib

    if "antenv.axon_hooks" in _sys.modules:
        return
    try:
        lib = ctypes.CDLL("/opt/axon/libaxon_pjrt.so")
        lib.axon_start_nrt_profile.argtypes = [
            ctypes.POINTER(ctypes.c_int64), ctypes.c_size_t,
        ]
        lib.axon_start_nrt_profile.restype = ctypes.c_int64
        lib.axon_stop_nrt_profile.argtypes = [ctypes.c_char_p]
        lib.axon_stop_nrt_profile.restype = ctypes.c_int64
    except (OSError, AttributeError):
        return

    @contextlib.contextmanager
    def _hook(output_dir, device_ids):
        import jax

        jax.devices()
        if device_ids:
            ids = (ctypes.c_int64 * len(device_ids))(*device_ids)
            rc = lib.axon_start_nrt_profile(ids, len(device_ids))
        else:
            rc = lib.axon_start_nrt_profile(None, 0)
        if rc != 0:
            raise RuntimeError(f"axon_start_nrt_profile rc={rc}")
        try:
            yield
        finally:
            n = lib.axon_stop_nrt_profile(str(output_dir).encode())
            if n < 0:
                raise RuntimeError(f"axon_stop_nrt_profile rc={n}")

    mod = types.ModuleType("antenv.axon_hooks")
    mod.get_axon_ntff_profile_hook = lambda: _hook
    mod.set_axon_ntff_profile_hook = lambda h: None
    _sys.modules["antenv.axon_hooks"] = mod
    # artifact upload has no bucket in this container; keep files local
    import concourse.bass_utils as _bu

    _bu.upload_artifacts = lambda d: d


def kernel(**inputs):
    from concourse.bass_utils import run_bass_kernel_spmd
    import ml_dtypes
    import os

    nc = _build()

    f32 = np.float32
    bf16 = ml_dtypes.bfloat16
    x = np.asarray(inputs["x"], dtype=f32).reshape(B, CIN, N).astype(bf16)
    y = np.asarray(inputs["y"], dtype=f32).reshape(B, CIN, N).astype(bf16)
    w_in = np.asarray(inputs["w_in"], dtype=f32)
    b_in = np.asarray(inputs["b_in"], dtype=f32).reshape(P, 1)
    ca_wq = np.asarray(inputs["ca_wq"], dtype=f32)
    ca_wk = np.asarray(inputs["ca_wk"], dtype=f32)
    ca_wv = np.asarray(inputs["ca_wv"], dtype=f32)
    g1 = np.asarray(inputs["ca_gamma"], dtype=f32).reshape(-1)[0]
    sa_wq = np.asarray(inputs["sa_wq"], dtype=f32)
    sa_wk = np.asarray(inputs["sa_wk"], dtype=f32)
    sa_wv = np.asarray(inputs["sa_wv"], dtype=f32)
    g2 = np.asarray(inputs["sa_gamma"], dtype=f32).reshape(-1)[0]
    conv1_w = np.asarray(inputs["conv1_w"], dtype=f32)
    conv2_w = np.asarray(inputs["conv2_w"], dtype=f32)
    bn1s = np.asarray(inputs["bn1_s"], dtype=f32).reshape(P, 1)
    bn1b = np.asarray(inputs["bn1_b"], dtype=f32).reshape(P, 1)
    bn2s = np.asarray(inputs["bn2_s"], dtype=f32).reshape(P, 1)
    bn2b = np.asarray(inputs["bn2_b"], dtype=f32).reshape(P, 1)
    pred_w = np.asarray(inputs["pred_w"], dtype=f32)
    pred_b = np.asarray(inputs["pred_b"], dtype=f32).reshape(1, 1)

    # host-side weight prep (small, O(C^2))
    w_inT = np.ascontiguousarray(
        w_in.T.reshape(NKO, P, P).transpose(1, 0, 2)
    ).astype(bf16)  # [cin_p, ko, cout]
    A1 = np.ascontiguousarray(ca_wq.T @ ca_wk)
    A2 = np.ascontiguousarray(sa_wq.T @ sa_wk)
    # conv taps: [o, i, 3, 3] -> lhsT per tap [i, o]; layout [i_p, tap, o]
    w1T = np.ascontiguousarray(
        conv1_w.transpose(2, 3, 1, 0).reshape(9, P, P).transpose(1, 0, 2)
    ).astype(bf16)
    w2T = np.ascontiguousarray(
        conv2_w.transpose(2, 3, 1, 0).reshape(9, P, P).transpose(1, 0, 2)
    ).astype(bf16)
    predT = np.zeros((P, P), f32)
    predT[:, 0] = pred_w[0]

    shared = {
        "w_inT": w_inT, "b_in": b_in, "A1": A1,
        "gwv1T": np.ascontiguousarray(g1 * ca_wv.T).astype(bf16),
        "A2": A2, "gwv2T": np.ascontiguousarray(g2 * sa_wv.T).astype(bf16),
        "w1T": w1T, "bn1s": bn1s,
        "bn1b": bn1b, "w2T": w2T, "bn2s": bn2s, "bn2b": bn2b,
        "predT": predT,
    }
    in_maps = [
        {"x": np.ascontiguousarray(x[i]), "y": np.ascontiguousarray(y[i]), **shared}
        for i in range(B)
    ]

    trace = bool(int(os.environ.get("KERNEL_TRACE", "0")))
    if trace:
        _install_ntff_hook()
    res = run_bass_kernel_spmd(nc, in_maps, core_ids=list(range(B)), trace=trace)
    if trace:
        _NC_CACHE["last_results"] = res
    out = np.stack(
        [res.results[i]["out"].reshape(1, H, W) for i in range(B)]
    ).astype(f32)
    return out + pred_b[0, 0]
